# revision 15
# baseline (speedup 1.0000x reference)
"""AttentionBlock (GroupNorm + single-head attention over HW tokens + proj +
residual) as a Bass/Tile kernel for 8 Trainium2 NeuronCores.

Sharding: data-parallel over batch B=32 -> 4 samples per core; 1x1-conv
weights replicated.

v2: fp8e4 (e4m3) DoubleRow matmuls for qkv/scores/denominators/attnout
(K=256 contracted per instruction at 0.5 cyc/row = 4x the fp32r rate);
proj stays fp32r for accuracy. Power-of-2 scales keep every fp8 operand in
the healthy e4m3 range and are folded into weight prep (host), the
PSUM->SBUF eviction scale, or the Exp activation's input scale/bias.
Softmax runs without max-subtraction (scores O(7); exp shifted by -2.77 so
p8 stays < 240); denominators via all-ones fp8 DR matmuls; the reciprocal
is multiplied into the attnout eviction (no separate t-pass); v/proj biases
fold into bp' = b_proj + w_proj^T b_v on the host; the +x residual is added
by the PE via a bf16 identity matmul into the proj PSUM so the final
eviction is a plain paired copy.

Engine split per sample: ACT = 8x exp on paired [128,2,512] PSUM tiles +
q evictions; DVE = k/v/o evictions, ao*rb, reciprocal; GPSIMD = GroupNorm
stat passes over x (also emitting the bf16 x copy used for the residual
matmul), the GroupNorm scalar chain, and the h=(x*sc+sh)*16 fp8 apply.
Sample 0 (kernel-start critical path) runs stats/GN-chain/h on ACT+DVE
instead. Both projections + output evictions are deferred until after both
query halves' score/exp blocks so the in-order PE queue never head-blocks
the next half's score matmuls on the softmax reciprocal chain.
"""

import numpy as np
import ml_dtypes

import concourse.bacc as bacc
import concourse.tile as tile
import concourse.mybir as mybir
from concourse.bass_utils import run_bass_kernel_spmd

F32 = mybir.dt.float32
F32R = mybir.dt.float32r
BF16 = mybir.dt.bfloat16
F8 = mybir.dt.float8e4
ALU = mybir.AluOpType
ACTF = mybir.ActivationFunctionType
DR = mybir.MatmulPerfMode.DoubleRow

N_CORES = 8
B, C, H, W = 32, 256, 32, 32
HW = H * W          # 1024
S = B // N_CORES    # 4 samples per core
G = 8               # groups
CG = C // G         # 32 channels per group
EPS = 1e-5
NC2 = C // 128      # channel chunks of 128
NH2 = HW // 512     # hw halves of 512

# power-of-2 scale ledger (see prep_inputs):
#   h8 = h * 2^4          (folded into maskTg / beta on host)
#   w8 q-section * 2^13 (after 1/sqrt(C) fold), k * 2^9, v * 2^9
#   q8 = q * 2^6  -> evict scale 2^-11
#   k8 = k * 2^2  -> evict scale 2^-11
#   v8 = v * 2^4  -> evict scale 2^-9
#   scores psum = s * 2^8 -> exp scale 2^-8, bias -2.77
#   aot = ao_psum * rb = attnout * 2^4 ; wpT * 2^-4 on host -> pp = proj
EXP_SHIFT = -2.77
Q_EVICT = float(2.0 ** -11)
K_EVICT = float(2.0 ** -11)
V_EVICT = float(2.0 ** -9)
EXP_SCALE = float(2.0 ** -8)


def _emit_xdma(nc, pools, s, x_ap, idx):
    """DMA one sample's x into SBUF (prefetched ahead of its stats pass).
    idx is the ring position (sample index within the rep sequence)."""
    sb, ps1, ps2 = pools
    xt = []
    for ci in range(NC2):
        x_t = sb.tile([128, HW], F32, name=f"x_i{idx}c{ci}", tag=f"x{ci}",
                      bufs=S)
        if idx == 0:
            nc.sync.dma_start(x_t[:, 0:512],
                              x_ap[s, ci * 128:(ci + 1) * 128, 0:512])
            nc.sync.dma_start(x_t[:, 512:HW],
                              x_ap[s, ci * 128:(ci + 1) * 128, 512:HW])
        else:
            nc.sync.dma_start(x_t[:], x_ap[s, ci * 128:(ci + 1) * 128, :])
    
        xt.append(x_t)
    return xt


def _emit_stats(nc, pools, wt, s, xt, post_x_cb=None):
    """Load x, produce per-channel scale/shift columns for GroupNorm and the
    bf16 copy of x used by the residual matmul. Steady-state stat passes and
    the scalar chain run on GPSIMD; sample 0 uses ACT+DVE (start latency).
    Group reduction via tiny fp32 matmuls; rstd via Newton rsqrt."""
    sb, ps1, ps2 = pools
    I32 = mybir.dt.int32
    ve = nc.vector

    xb, st = [], []
    if post_x_cb is not None:
        post_x_cb()
    for ci in range(NC2):
        x_t = xt[ci]
        st_t = sb.tile([128, 2], F32, name=f"st_s{s}c{ci}", tag=f"st{ci}", bufs=S)
        xb_t = sb.tile([128, HW], BF16, name=f"xb_s{s}c{ci}", tag=f"xb{ci}",
                       bufs=3)
        xb.append(xb_t)
        st.append(st_t)
        if s == 0:
            # halves so stats overlap the split x DMA; chunk0 on ACT,
            # chunk1 on DVE
            p4 = sb.tile([128, 4], F32, name=f"p4_s{s}c{ci}", tag=f"p4{ci}",
                         bufs=1)
            for hh in range(2):
                xs = x_t[:, hh * 512:(hh + 1) * 512]
                xbs = xb_t[:, hh * 512:(hh + 1) * 512]
                if ci == 0:
                    nc.scalar.activation(xbs, xs, ACTF.Copy,
                                         accum_out=p4[:, hh:hh + 1])
                    scr2 = sb.tile([128, 512], F8, name=f"sq_s{s}c{ci}e{hh}",
                                   tag="sqscr", bufs=2)
                    nc.scalar.activation(scr2[:], xs, ACTF.Square,
                                         accum_out=p4[:, 2 + hh:3 + hh])
                else:
                    nc.vector.tensor_scalar(xbs, xs, 0.0, None, op0=ALU.add,
                                            op1=ALU.add,
                                            accum_out=p4[:, hh:hh + 1])
                    scr3 = sb.tile([128, 512], F8, name=f"sq_s{s}c{ci}e{hh}",
                                   tag="sqscr", bufs=2)
                    nc.vector.scalar_tensor_tensor(
                        scr3[:], in0=xs, scalar=0.0, in1=xs, op0=ALU.add,
                        op1=ALU.mult, accum_out=p4[:, 2 + hh:3 + hh])
            nc.vector.tensor_add(st_t[:, 0:1], p4[:, 0:1], p4[:, 1:2])
            nc.vector.tensor_add(st_t[:, 1:2], p4[:, 2:3], p4[:, 3:4])
        else:
            sq_t = sb.tile([128, HW], F8, name=f"sq_s{s}c{ci}", tag="sqscr",
                           bufs=2)
            nc.vector.tensor_scalar(xb_t[:], x_t[:], 0.0, None, op0=ALU.add,
                                    op1=ALU.add, accum_out=st_t[:, 0:1])
            nc.vector.scalar_tensor_tensor(sq_t[:], in0=x_t[:], scalar=0.0,
                                           in1=x_t[:], op0=ALU.add,
                                           op1=ALU.mult,
                                           accum_out=st_t[:, 1:2])

    # group stats: gst = [mean, ex2] (gmask carries 1/n)
    gst = ps1.tile([8, 2], F32, name=f"gst_s{s}", tag="sm", bufs=2)
    for ci in range(NC2):
        nc.tensor.matmul(gst[:], wt["gmask"][:, ci * G:(ci + 1) * G], st[ci][:],
                         start=(ci == 0), stop=(ci == NC2 - 1))
    gsb = sb.tile([8, 2], F32, name=f"gsb_s{s}", tag="gsb", bufs=2)
    nc.vector.tensor_copy(gsb[:], gst[:])
    msq = sb.tile([8, 1], F32, name=f"msq_s{s}", tag="msq", bufs=2)
    ve.tensor_mul(msq[:], gsb[:, 0:1], gsb[:, 0:1])
    var = sb.tile([8, 1], F32, name=f"var_s{s}", tag="var", bufs=2)
    nc.vector.scalar_tensor_tensor(var[:], in0=gsb[:, 1:2], scalar=EPS,
                                   in1=msq[:], op0=ALU.add, op1=ALU.subtract)
    # rstd = rsqrt(var): fast-inverse-sqrt bit trick + 2 Newton steps
    ish = sb.tile([8, 1], I32, name=f"ish_s{s}", tag="ish", bufs=2)
    ve.tensor_scalar(ish[:], var[:].bitcast(I32), 1, None,
                     op0=ALU.arith_shift_right)
    yib = sb.tile([8, 1], I32, name=f"yib_s{s}", tag="yib", bufs=2)
    ve.tensor_tensor(yib[:], wt["magic"][0:8, :].bitcast(I32), ish[:],
                     op=ALU.subtract)
    y = yib[:].bitcast(F32)
    for it in range(2):
        ta = sb.tile([8, 1], F32, name=f"ta{it}_s{s}", tag=f"ta{it}", bufs=2)
        ve.tensor_mul(ta[:], y, y)
        tb = sb.tile([8, 1], F32, name=f"tb{it}_s{s}", tag=f"tb{it}", bufs=2)
        ve.tensor_mul(tb[:], ta[:], var[:])
        tcr = sb.tile([8, 1], F32, name=f"tc{it}_s{s}", tag=f"tc{it}", bufs=2)
        ve.tensor_scalar(tcr[:], tb[:], -0.5, 1.5, op0=ALU.mult, op1=ALU.add)
        yn = sb.tile([8, 1], F32, name=f"yn{it}_s{s}", tag=f"yn{it}", bufs=2)
        ve.tensor_mul(yn[:], y, tcr[:])
        y = yn[:]
    # gv2 = [rstd, mean*rstd] feeds the (16*gamma)-scaled broadcast matmul
    gv2 = sb.tile([8, 2], F32, name=f"gv2_s{s}", tag="gv2", bufs=2)
    ve.tensor_copy(gv2[:, 0:1], y)
    ve.tensor_mul(gv2[:, 1:2], y, gsb[:, 0:1])

    scc, shc = [], []
    for ci in range(NC2):
        # mr = [16*gamma*rstd, 16*gamma*mean*rstd] per channel
        mr = ps1.tile([128, 2], F32, name=f"mr_s{s}c{ci}", tag="sm", bufs=2)
        nc.tensor.matmul(mr[:], wt["maskTg"][:, ci * 128:(ci + 1) * 128],
                         gv2[:], start=True, stop=True)
        sh_t = sb.tile([128, 1], F32, name=f"sh_s{s}c{ci}", tag=f"sh{ci}",
                       bufs=S)
        nc.vector.tensor_sub(sh_t[:], wt["beta"][ci], mr[:, 1:2])
        shc.append(sh_t)
        sc_t = sb.tile([128, 1], F32, name=f"scc_s{s}c{ci}",
                       tag=f"scc{ci}", bufs=S)
        nc.vector.tensor_copy(sc_t[:], mr[:, 0:1])
        scc.append(sc_t)
    return xt, xb, scc, shc


def _emit_h(nc, pools, wt, s, stats):
    """h8 = (x*scc + shc) in fp8 at 2^4 scale (scales host-folded). GPSIMD
    (SBUF->SBUF); sample 0 on DVE for start latency. h8 layout
    [128, 2(c-chunk), HW] for DoubleRow."""
    sb, ps1, ps2 = pools
    xt, xb, scc, shc = stats
    ve = nc.vector if s == 0 else nc.gpsimd
    h8 = sb.tile([128, NC2, HW], F8, name=f"h8_s{s}", tag="h8", bufs=2)
    if s == 0:
        for hh in range(2):
            hs = slice(hh * 512, (hh + 1) * 512)
            for ci in range(NC2):
                ve.tensor_scalar(h8[:, ci, hs], xt[ci][:, hs], scc[ci][:],
                                 shc[ci][:], op0=ALU.mult, op1=ALU.add)
    else:
        for ci in range(NC2):
            ve.tensor_scalar(h8[:, ci, :], xt[ci][:], scc[ci][:],
                             shc[ci][:], op0=ALU.mult, op1=ALU.add)
    return xt, xb, h8


def _emit_qkv(nc, pools, wt, s, h8):
    """QKV matmuls + fp8 evictions for sample s."""
    sb, ps1, ps2 = pools
    q8 = sb.tile([128, NC2, HW], F8, name=f"q8_s{s}", tag="q8", bufs=2)
    k8 = sb.tile([128, NC2, HW], F8, name=f"k8_s{s}", tag="k8", bufs=2)
    for ih in range(NH2):
        hs = slice(ih * 512, (ih + 1) * 512)
        qp = ps2.tile([128, 2, 512], F32, name=f"qp_s{s}h{ih}", tag="p2", bufs=3)
        for ci in range(NC2):
            nc.tensor.matmul(qp[:, ci, :],
                             wt["w8q"][:, :, ci * 128:(ci + 1) * 128],
                             h8[:, :, hs], start=True, stop=True,
                             perf_mode=DR)
        nc.scalar.activation(q8[:, :, hs], qp[:], ACTF.Copy, scale=Q_EVICT)
        kp = ps2.tile([128, 2, 512], F32, name=f"kp_s{s}h{ih}", tag="p2", bufs=3)
        for ci in range(NC2):
            nc.tensor.matmul(kp[:, ci, :],
                             wt["w8q"][:, :, C + ci * 128:C + (ci + 1) * 128],
                             h8[:, :, hs], start=True, stop=True,
                             perf_mode=DR)
        nc.vector.tensor_scalar(k8[:, :, hs], kp[:], K_EVICT, None,
                                op0=ALU.mult)

    # v8 in [128, 8(j-chunk), 256(c)] fp8; PSUM pairs hold 4 j-chunks.
    v8 = sb.tile([128, HW // 128, C], F8, name=f"v8_s{s}", tag="v8", bufs=2)
    for vh in range(NH2):
        vp = ps2.tile([128, 2, 512], F32, name=f"vp_s{s}v{vh}", tag="p2", bufs=3)
        for u in range(2):
            for sub in range(2):
                j = 4 * vh + 2 * u + sub
                nc.tensor.matmul(vp[:, u, sub * C:(sub + 1) * C],
                                 h8[:, :, j * 128:(j + 1) * 128],
                                 wt["w8q"][:, :, 2 * C:3 * C],
                                 start=(sub == 0), stop=(sub == 1),
                                 perf_mode=DR)
        nc.vector.tensor_scalar(v8[:, 4 * vh:4 * vh + 4, :], vp[:], V_EVICT,
                                None, op0=ALU.mult)
    return q8, k8, v8


def _emit_scores(nc, pools, wt, s, ih, q8, k8, v8):
    """Scores + exp + denominators + attnout (with 1/denominator folded into
    the eviction) for one query half. Returns the two aot tiles."""
    sb, ps1, ps2 = pools
    hs = slice(ih * 512, (ih + 1) * 512)
    pt = []
    for m in range(4):
        sp = ps2.tile([128, 2, 512], F32, name=f"sp_s{s}h{ih}m{m}",
                      tag="p2", bufs=3)
        for u in range(2):
            j = 2 * m + u
            nc.tensor.matmul(sp[:, u, :],
                             k8[:, :, j * 128:(j + 1) * 128],
                             q8[:, :, hs], start=True, stop=True,
                             perf_mode=DR)
        p_t = sb.tile([128, 2, 512], F8, name=f"p_s{s}h{ih}m{m}", tag="pt",
                      bufs=8)
        nc.scalar.activation(p_t[:], sp[:], ACTF.Exp,
                             bias=wt["eshift"], scale=EXP_SCALE)
        pt.append(p_t)
    # softmax denominators: all-ones DR matmuls broadcast the partition
    # reduction to all 128 partitions
    dn = ps1.tile([128, 512], F32, name=f"dn_s{s}h{ih}", tag="sm", bufs=2)
    for m in range(4):
        nc.tensor.matmul(dn[:], wt["ones8"][:], pt[m][:],
                         start=(m == 0), stop=(m == 3), perf_mode=DR)
    rb_t = sb.tile([128, 512], F32, name=f"rb_s{s}h{ih}", tag="rb", bufs=2)
    nc.vector.reciprocal(rb_t[:], dn[:])

    ao_sb = []
    for ci in range(NC2):
        ao = ps1.tile([128, 512], F32, name=f"ao_s{s}h{ih}c{ci}", tag="sm",
                      bufs=2)
        for m in range(4):
            nc.tensor.matmul(ao[:],
                             v8[:, 2 * m:2 * m + 2, ci * 128:(ci + 1) * 128],
                             pt[m][:], start=(m == 0), stop=(m == 3),
                             perf_mode=DR)
        ao_t = sb.tile([128, 512], F32R, name=f"aot_s{s}h{ih}c{ci}",
                       tag="aot", bufs=4)
        nc.vector.tensor_tensor(ao_t[:], ao[:], rb_t[:], op=ALU.mult)
        ao_sb.append(ao_t)
    return ao_sb


def _emit_proj_mm(nc, pools, wt, s, ih, ao_sb, xb):
    """Projection matmuls (fp32r) + x residual (bf16 identity matmul) into
    one PSUM pair. Eviction is emitted separately (_emit_proj_out) so the
    DVE queue ordering is decoupled from the PE ordering."""
    sb, ps1, ps2 = pools
    hs = slice(ih * 512, (ih + 1) * 512)
    pp = ps2.tile([128, 2, 512], F32, name=f"pp_s{s}h{ih}", tag="p2", bufs=3)
    for co in range(NC2):
        for cc in range(NC2):
            nc.tensor.matmul(pp[:, co, :],
                             wt["wp"][cc][:, co * 128:(co + 1) * 128],
                             ao_sb[cc][:], start=(cc == 0), stop=False)
        nc.tensor.matmul(pp[:, co, :], wt["ibf"][:], xb[co][:, hs],
                         start=False, stop=True)
    return pp


def _emit_proj_out(nc, pools, s, ih, pp, out_ap):
    sb, ps1, ps2 = pools
    hs = slice(ih * 512, (ih + 1) * 512)
    o_t = sb.tile([128, 2, 512], F32, name=f"o_s{s}h{ih}", tag="o", bufs=3)
    nc.scalar.activation(o_t[:], pp[:], ACTF.Copy)
    for co in range(NC2):
        nc.sync.dma_start(out_ap[s, co * 128:(co + 1) * 128, hs],
                          o_t[:, co, :])


def build_program(reps=1):
    nc = bacc.Bacc("TRN2", target_bir_lowering=False, debug=False,
                   enable_asserts=False, num_devices=N_CORES)

    x_ap = nc.dram_tensor("x", [S, C, HW], F32, kind="ExternalInput").ap()
    w8q_ap = nc.dram_tensor("w8q", [128, NC2, 3 * C], F8,
                            kind="ExternalInput").ap()
    wp_ap = nc.dram_tensor("wpT", [C, C], F32R, kind="ExternalInput").ap()
    ca_ap = nc.dram_tensor("constsA", [128, NC2 * G + 4], F32,
                           kind="ExternalInput").ap()
    gmt_ap = nc.dram_tensor("gmaskTg", [G, C], F32, kind="ExternalInput").ap()
    ones_ap = nc.dram_tensor("ones8", [128, 2, 128], F8,
                             kind="ExternalInput").ap()
    ibf_ap = nc.dram_tensor("ibf", [128, 128], BF16, kind="ExternalInput").ap()
    out_ap = nc.dram_tensor("out", [S, C, HW], F32, kind="ExternalOutput").ap()

    with tile.TileContext(nc) as tc:
        with (
            tc.tile_pool(name="wpool", bufs=1) as wp,
            tc.tile_pool(name="sb", bufs=2) as sb,
            tc.tile_pool(name="ps1", bufs=2, space="PSUM") as ps1,
            tc.tile_pool(name="ps2", bufs=3, space="PSUM") as ps2,
        ):
            # stats-critical constants first in ONE small DMA
            constsA = wp.tile([128, NC2 * G + 4], F32, name="constsA",
                              tag="constsA")
            nc.sync.dma_start(constsA[:], ca_ap[:])
            gmask = constsA[:, 0:NC2 * G]
            cvec = constsA[:, NC2 * G:]

            maskTg = wp.tile([G, C], F32, name="maskTg", tag="maskTg")
            ones8 = wp.tile([128, 2, 128], F8, name="ones8", tag="ones8")
            ibf = wp.tile([128, 128], BF16, name="ibf", tag="ibf")
            wt = {
                "gmask": gmask,
                "maskTg": maskTg,
                "ones8": ones8,
                "ibf": ibf,
                "beta": [cvec[:, ci:ci + 1] for ci in range(NC2)],
                "magic": cvec[:, 2:3],
                "eshift": cvec[:, 3:4],
            }

            pools = (sb, ps1, ps2)
            seq = [(rep, s) for rep in range(reps) for s in range(S)]
            n_seq = len(seq)

            xts = {0: _emit_xdma(nc, pools, 0, x_ap, 0)}
            stats0 = _emit_stats(
                nc, pools, wt, 0, xts[0],
                post_x_cb=lambda: nc.sync.dma_start(maskTg[:], gmt_ap[:]))
            nc.sync.dma_start(ones8[:], ones_ap[:])
            nc.sync.dma_start(ibf[:], ibf_ap[:])
            if n_seq > 1:
                xts[1] = _emit_xdma(nc, pools, seq[1][1], x_ap, 1)
            stats_ring = {0: stats0}
            if n_seq > 1:
                stats_ring[1] = _emit_stats(nc, pools, wt, seq[1][1], xts[1])

            # big weights after sample 0/1 x DMAs are in flight
            w8q = wp.tile([128, NC2, 3 * C], F8, name="w8q", tag="w8q")
            nc.sync.dma_start(w8q[:], w8q_ap[:])
            wp0 = wp.tile([128, C], F32R, name="wp0", tag="wp0")
            nc.sync.dma_start(wp0[:], wp_ap[0:128, :])
            wp1 = wp.tile([128, C], F32R, name="wp1", tag="wp1")
            nc.sync.dma_start(wp1[:], wp_ap[128:256, :])
            wt["w8q"] = w8q
            wt["wp"] = [wp0, wp1]
            if n_seq > 2:
                xts[2] = _emit_xdma(nc, pools, seq[2][1], x_ap, 2)

            cur = _emit_h(nc, pools, wt, 0, stats_ring.pop(0))
            qkv_cur = _emit_qkv(nc, pools, wt, 0, cur[2])

            pending = None
            for i in range(n_seq):
                s = seq[i][1]
                xt, xb, h8 = cur
                q8, k8, v8 = qkv_cur

                ao0 = _emit_scores(nc, pools, wt, s, 0, q8, k8, v8)
                # previous sample's second-half projection lands here so the
                # in-order PE/DVE queues never wait on its reciprocal chain
                if pending is not None:
                    pp1 = _emit_proj_mm(nc, pools, wt, *pending)
                    _emit_proj_out(nc, pools, pending[0], 1, pp1, out_ap)
                    pending = None

                # mid-sample: prefetch x(i+3), normalize h(i+1),
                # stats for sample i+2 (engine: GPSIMD)
                if i + 3 < n_seq:
                    xts[i + 3] = _emit_xdma(nc, pools, seq[i + 3][1], x_ap,
                                            i + 3)
                nxt = None
                if i + 1 < n_seq:
                    nxt = _emit_h(nc, pools, wt, seq[i + 1][1],
                                  stats_ring.pop(i + 1))
                if i + 2 < n_seq:
                    stats_ring[i + 2] = _emit_stats(nc, pools, wt,
                                                    seq[i + 2][1],
                                                    xts.pop(i + 2))

                ao1 = _emit_scores(nc, pools, wt, s, 1, q8, k8, v8)

                # next sample's QKV before this sample's projections so the
                # in-order PE queue never waits on the reciprocal chain
                if i + 1 < n_seq:
                    qkv_cur = _emit_qkv(nc, pools, wt, seq[i + 1][1], nxt[2])

                pp0 = _emit_proj_mm(nc, pools, wt, s, 0, ao0, xb)
                _emit_proj_out(nc, pools, s, 0, pp0, out_ap)
                pending = (s, 1, ao1, xb)
                cur = nxt
            if pending is not None:
                pp1 = _emit_proj_mm(nc, pools, wt, *pending)
                _emit_proj_out(nc, pools, pending[0], 1, pp1, out_ap)

    nc.compile()
    return nc


def prep_inputs(x, gamma, beta, w_qkv, b_qkv, w_proj, b_proj):
    """Host-side prep: shard x over cores, quantize/scale/transpose weights."""
    E4 = ml_dtypes.float8_e4m3
    x = np.ascontiguousarray(x, dtype=np.float32).reshape(B, C, HW)
    x_shards = x.reshape(N_CORES, S, C, HW)

    scale = np.float32(1.0 / np.sqrt(np.float32(C)))
    wqkvT = np.ascontiguousarray(np.asarray(w_qkv, np.float32).T)  # (C, 3C)
    wqkvT[:, 0:C] *= scale * np.float32(2.0 ** 13)
    wqkvT[:, C:2 * C] *= np.float32(2.0 ** 9)
    wqkvT[:, 2 * C:3 * C] *= np.float32(2.0 ** 9)
    # w8q[p, cc, o] = wqkvT[cc*128 + p, o]
    w8q = np.ascontiguousarray(
        wqkvT.reshape(NC2, 128, 3 * C).transpose(1, 0, 2)).astype(E4)

    b_qkv = np.asarray(b_qkv, np.float32)
    assert np.all(b_qkv[:2 * C] == 0.0), "nonzero q/k bias not supported"
    bv = b_qkv[2 * C:3 * C]
    w_proj = np.asarray(w_proj, np.float32)
    bp_f = np.asarray(b_proj, np.float32) + w_proj @ bv
    assert np.all(bp_f == 0.0), "nonzero proj/v bias not supported"
    wpT = np.ascontiguousarray(w_proj.T * np.float32(2.0 ** -4))

    gam = np.asarray(gamma, np.float32)
    bet = np.asarray(beta, np.float32).reshape(NC2, 128)
    cvec = np.zeros((128, 4), np.float32)
    for ci in range(NC2):
        cvec[:, ci] = bet[ci] * np.float32(16.0)
    cvec[:, 2] = np.uint32(0x5F3759DF).view(np.float32)
    cvec[:, 3] = np.float32(EXP_SHIFT)

    inv_n = np.float32(1.0 / (CG * HW))
    gmask = np.zeros((128, NC2 * G), np.float32)
    gmaskTg = np.zeros((G, C), np.float32)
    for c in range(C):
        g = c // CG
        gmaskTg[g, c] = gam[c] * np.float32(16.0)
        gmask[c % 128, (c // 128) * G + g] = inv_n

    shared = {
        "w8q": w8q,
        "wpT": wpT,
        "constsA": np.ascontiguousarray(
            np.concatenate([gmask, cvec], axis=1)),
        "gmaskTg": gmaskTg,
        "ones8": np.ones((128, 2, 128), E4),
        "ibf": np.eye(128, dtype=ml_dtypes.bfloat16),
    }
    return [dict(shared, x=np.ascontiguousarray(x_shards[i]))
            for i in range(N_CORES)]


_NC_CACHE = {}


def kernel(x, gamma, beta, w_qkv, b_qkv, w_proj, b_proj):
    if "nc" not in _NC_CACHE:
        _NC_CACHE["nc"] = build_program()
    nc = _NC_CACHE["nc"]
    in_maps = prep_inputs(x, gamma, beta, w_qkv, b_qkv, w_proj, b_proj)
    res = run_bass_kernel_spmd(nc, in_maps, list(range(N_CORES)))
    out = np.stack([res.results[i]["out"] for i in range(N_CORES)])
    return out.reshape(B, C, H, W)
